# revision 12
# baseline (speedup 1.0000x reference)
"""Trainium2 Bass kernel v3 for nn_DNM_Conv_fold.

Math (same folding as baseline, all validated):
  out[px, o] = a[px] * sum_m relu( (Wc^T x)[px, mo] + sv[px]*bq[mo] )
  Wc = gamma-folded W, rows centered  (makes LN mean-subtraction implicit)
  sv = sqrt(var+eps) rides as contraction row 65; bq = beta@Weff - q
  a  = 1/sv applied INSIDE the relu evac (relu(a*z) = a*relu(z), a>0)

v3 structure (px-major, bf16):
  - x, W, relu outputs, m-sums, HBM output all bf16 (DVE 2x, half HBM)
  - stats: x restacked [128, chunk/2]; squares on GPSIMD; 4-way col-tiled
    matmuls (tile_position (0,32j)) -> mu/e2 psum rows {32j,32j+1};
    batch-safe math (max(var,0)+eps guards garbage rows)
  - sv -> aug row 64 of xt via 4 reshape DMAs
  - a  -> column form via 8 row DMAs + one PE transpose -> a_cols [128,32]
  - main matmul per 128-px tile: lhsT = xt[:,tile] (K=65), rhs = wc [65,256],
    psum tile = own full bank; evac = relu(a*z): ACT activation(scale=) or
    DVE tensor_scalar(mult,max), 28/4 split
  - msum: two batched free-dim bf16 adds (256->128->64)
  - out: [px, 64] bf16 HBM layout, host does final transpose to [B,O,H,W]

Sharding: 8 cores; core k = batch k//2, pixel half k%2 (73728 px each).
"""

import sys

sys.path.insert(0, "/opt/trn_rl_repo")

import numpy as np
import ml_dtypes

# ---- problem constants ----
B, C, O, M, H, Wd = 4, 64, 64, 4, 384, 384
EPS = 1e-5
MO = M * O  # 256
NCORES = 8
PIX_PER_CORE = B * H * Wd // NCORES  # 73728
CHUNK = 4096
NCHUNK = PIX_PER_CORE // CHUNK  # 18
NT = CHUNK // 128  # 32 px-tiles per chunk
DVE_EVAC = {3, 6, 9, 12, 15, 19, 23, 27, 31}  # px-tiles evacuated by DVE

_cache = {}


def _build(pix_per_core=PIX_PER_CORE, chunk=CHUNK, repeat=1):
    import contextlib

    from concourse import bacc, bass, tile

    mybir = bass.mybir
    f32 = mybir.dt.float32
    bf16 = mybir.dt.bfloat16
    AF = mybir.ActivationFunctionType
    ALU = mybir.AluOpType

    nchunk = pix_per_core // chunk
    nt = chunk // 128
    half = chunk // 2  # stacked width

    nc = bacc.Bacc(None, target_bir_lowering=False)
    xin = nc.declare_dram_parameter("xin", [C, pix_per_core], bf16, isOutput=False)
    wc_d = nc.declare_dram_parameter("wc", [C + 1, MO], bf16, isOutput=False)
    cst_d = nc.declare_dram_parameter("cst", [128, 2], bf16, isOutput=False)
    id_d = nc.declare_dram_parameter("ident", [32, 32], f32, isOutput=False)
    sel_d = nc.declare_dram_parameter("sel", [128, 8], bf16, isOutput=False)
    out_d = nc.declare_dram_parameter("out", [pix_per_core, O], bf16, isOutput=True)

    with tile.TileContext(nc) as tc:
        with (
            tc.tile_pool(name="const", bufs=1) as constp,
            tc.tile_pool(name="xtp", bufs=2) as xtp,
            tc.tile_pool(name="xsp", bufs=2) as xsp,
            tc.tile_pool(name="sqp", bufs=2) as sqp,
            tc.tile_pool(name="stp", bufs=2) as stp,
            tc.tile_pool(name="acp", bufs=2) as acp,
            tc.tile_pool(name="rp", bufs=2) as rp,
            tc.tile_pool(name="s1p", bufs=2) as s1p,
            tc.tile_pool(name="s2p", bufs=2) as s2p,
            tc.tile_pool(name="ps_main", bufs=5, space="PSUM") as ps_mainp,
            tc.tile_pool(name="ps_stat", bufs=1, space="PSUM") as ps_statp,
            tc.tile_pool(name="ps_t", bufs=1, space="PSUM") as ps_tp,
        ):
            wc_sb = constp.tile([C + 1, MO], bf16)
            cst4 = constp.tile([128, 2], bf16)
            ident = constp.tile([32, 32], f32)
            sel = constp.tile([128, 8], bf16)
            epsb = constp.tile([128, 1], f32)
            nc.sync.dma_start(out=wc_sb[:, :], in_=wc_d[:, :])
            nc.sync.dma_start(out=cst4[:, :], in_=cst_d[:, :])
            nc.sync.dma_start(out=ident[:, :], in_=id_d[:, :])
            nc.sync.dma_start(out=sel[:, :], in_=sel_d[:, :])
            nc.gpsimd.memset(epsb[:, :], EPS)

            def load(ci):
                p0 = ci * chunk
                xt = xtp.tile([C + 1, chunk], bf16, tag="xt")
                nc.sync.dma_start(out=xt[0:C, :], in_=xin[:, p0 : p0 + chunk])
                # stacked copy loaded straight from DRAM (independent of xt)
                xs = xsp.tile([128, half], bf16, tag="xs")
                nc.sync.dma_start(
                    out=xs[:, :],
                    in_=xin[:, p0 : p0 + chunk].rearrange("c (g n) -> g c n", g=2),
                )
                return xt, xs

            def stats_part(ci, xt, xs):
                # ---- squares on GPSIMD
                sq = sqp.tile([128, half], bf16, tag="sq")
                nc.gpsimd.tensor_mul(sq[:, :], xs[:, :], xs[:, :])

                # ---- col-tiled stats matmuls: mu rows {32j,32j+1} etc
                smu = ps_statp.tile([98, 512], f32, tag="smu")
                se2 = ps_statp.tile([98, 512], f32, tag="se2")
                for j in range(4):
                    nc.tensor.matmul(
                        smu[32 * j : 32 * j + 2, :],
                        cst4[:, :],
                        xs[:, 512 * j : 512 * (j + 1)],
                        start=True,
                        stop=True,
                        tile_position=(0, 32 * j),
                    )
                for j in range(4):
                    nc.tensor.matmul(
                        se2[32 * j : 32 * j + 2, :],
                        cst4[:, :],
                        sq[:, 512 * j : 512 * (j + 1)],
                        start=True,
                        stop=True,
                        tile_position=(0, 32 * j),
                    )
                muT = stp.tile([98, 512], f32, tag="muT")
                e2T = stp.tile([98, 512], f32, tag="e2T")
                nc.scalar.activation(muT[:, :], smu[0:98, :], AF.Copy)
                nc.scalar.activation(e2T[:, :], se2[0:98, :], AF.Copy)

                # ---- batched stat math (garbage rows are guarded by max(.,0))
                musq = stp.tile([98, 512], f32, tag="musq")
                nc.vector.tensor_mul(musq[:, :], muT[:, :], muT[:, :])
                varr = stp.tile([98, 512], f32, tag="varr")
                nc.vector.scalar_tensor_tensor(
                    varr[:, :], musq[:, :], -1.0, e2T[:, :], ALU.mult, ALU.add
                )
                varm = stp.tile([98, 512], f32, tag="varm")
                nc.vector.tensor_scalar_max(varm[:, :], varr[:, :], 0.0)
                svf = stp.tile([98, 512], f32, tag="svf")
                nc.scalar.activation(
                    svf[:, :], varm[:, :], AF.Sqrt, bias=epsb[0:98, 0:1]
                )
                svb = stp.tile([98, 512], bf16, tag="svb")
                nc.vector.tensor_copy(svb[:, :], svf[:, :])
                af_ = stp.tile([98, 512], f32, tag="af")
                nc.vector.reciprocal_approx_fast(af_[:, :], svf[:, :])
                a_bf = stp.tile([98, 512], bf16, tag="a_bf")
                nc.vector.tensor_copy(a_bf[:, :], af_[:, :])
                return svb, a_bf

            def late_part(ci, xt, svb, a_bf):
                # ---- compact sv and a via selector matmuls (row 4g+j)
                svC = ps_tp.tile([128, 512], f32, tag="ps_misc", name="svC")
                nc.tensor.matmul(
                    svC[0:8, :], sel[0:98, :], svb[:, :], start=True, stop=True
                )
                svCs = stp.tile([8, 512], bf16, tag="svCs")
                nc.vector.tensor_copy(svCs[:, :], svC[0:8, :])
                aC = ps_tp.tile([128, 512], f32, tag="ps_misc", name="aC")
                nc.tensor.matmul(
                    aC[0:8, :], sel[0:98, :], a_bf[:, :], start=True, stop=True
                )
                aCs = stp.tile([8, 512], f32, tag="aCs")
                nc.vector.tensor_copy(aCs[:, :], aC[0:8, :])
                # sv -> aug row 64 of xt: ONE flatten DMA (flat order g,j,n)
                nc.sync.dma_start(out=xt[C : C + 1, :], in_=svCs[:, :])
                # a -> a32 [32, 128]: ONE flatten DMA, then PE transpose
                a32 = stp.tile([32, 128], f32, tag="a32")
                nc.sync.dma_start(out=a32[:, :], in_=aCs[:, :])
                ps_t = ps_tp.tile([128, 512], f32, tag="ps_misc", name="ps_t")
                nc.tensor.transpose(ps_t[:, 0:32], a32[:, :], ident[:, :])
                acols = acp.tile([128, 32], f32, tag="acols")
                nc.vector.tensor_copy(acols[:, :], ps_t[:, 0:32])
                return acols

            def main_part(ci, xt, acols):
                p0 = ci * chunk

                # ---- main matmuls + fused relu(a*z) evac
                rall = rp.tile([128, 256 * nt], bf16, tag="rall")
                for t in range(nt):
                    pt = ps_mainp.tile([128, 512], f32, tag="pt")
                    nc.tensor.matmul(
                        pt[:, 0:256],
                        xt[:, 128 * t : 128 * (t + 1)],
                        wc_sb[:, :],
                        start=True,
                        stop=True,
                    )
                    if t in DVE_EVAC:
                        nc.vector.tensor_scalar(
                            rall[:, 256 * t : 256 * (t + 1)],
                            pt[:, 0:256],
                            acols[:, t : t + 1],
                            0.0,
                            ALU.mult,
                            ALU.max,
                        )
                    else:
                        nc.scalar.activation(
                            rall[:, 256 * t : 256 * (t + 1)],
                            pt[:, 0:256],
                            AF.Relu,
                            scale=acols[:, t : t + 1],
                        )

                # ---- m-sum: 256 -> 128 -> 64, batched free-dim bf16 adds
                s1 = s1p.tile([128, 128 * nt], bf16, tag="s1")
                rv = rall[:, :].rearrange("p (t d) -> p t d", d=256)
                s1v = s1[:, :].rearrange("p (t d) -> p t d", d=128)
                nc.vector.tensor_add(s1v, rv[:, :, 0:128], rv[:, :, 128:256])
                s2 = s2p.tile([128, 64 * nt], bf16, tag="s2")
                s1r = s1[:, :].rearrange("p (t d) -> p t d", d=128)
                s2v = s2[:, :].rearrange("p (t d) -> p t d", d=64)
                nc.vector.tensor_add(s2v, s1r[:, :, 0:64], s1r[:, :, 64:128])

                # ---- out DMA: [px, 64] bf16
                nc.sync.dma_start(
                    out=out_d[p0 : p0 + chunk, :].rearrange("(t p) o -> p t o", p=128),
                    in_=s2[:, :].rearrange("p (t o) -> p t o", o=64),
                )

            rep_ctx = (
                tc.For_i(0, repeat, 1) if repeat > 1 else contextlib.nullcontext()
            )
            with rep_ctx:
                pend = load(0)
                st_cur = stats_part(0, *pend)
                for ci in range(nchunk):
                    cur = pend
                    if ci + 1 < nchunk:
                        pend = load(ci + 1)
                        st_next = stats_part(ci + 1, *pend)
                    acols = late_part(ci, cur[0], *st_cur)
                    main_part(ci, cur[0], acols)
                    if ci + 1 < nchunk:
                        st_cur = st_next
    nc.compile()
    return nc


def _host_consts(W, q, gamma, beta):
    W_eff = (W.astype(np.float32) * gamma.astype(np.float32)[None, None, :]).reshape(
        MO, C
    )
    Wc = W_eff - W_eff.mean(axis=1, keepdims=True, dtype=np.float32)
    bias = beta.astype(np.float32) @ W_eff.T  # [MO]
    bq = (bias - np.float32(q)).astype(np.float32)
    wc_aug = np.concatenate([Wc.T, bq[None, :]], axis=0)  # [65, 256]
    wc_aug = wc_aug.astype(ml_dtypes.bfloat16)
    cst4 = np.zeros((128, 2), np.float32)
    cst4[0:64, 0] = 1.0 / C
    cst4[64:128, 1] = 1.0 / C
    cst4 = cst4.astype(ml_dtypes.bfloat16)
    ident = np.eye(32, dtype=np.float32)
    sel = np.zeros((128, 8), np.float32)
    for g in range(2):
        for j in range(4):
            sel[32 * j + g, 4 * g + j] = 1.0
    sel = sel.astype(ml_dtypes.bfloat16)
    return wc_aug, cst4, ident, sel


def _prep_in_maps(inputs):
    x = np.ascontiguousarray(np.asarray(inputs["x"], dtype=np.float32))
    W = np.asarray(inputs["W"], dtype=np.float32)
    q = float(np.asarray(inputs["q"]).reshape(-1)[0])
    gamma = np.asarray(inputs["gamma"], dtype=np.float32)
    beta = np.asarray(inputs["beta"], dtype=np.float32)

    wc_aug, cst4, ident, sel = _host_consts(W, q, gamma, beta)

    xf = x.reshape(B, C, H * Wd)
    in_maps = []
    for k in range(NCORES):
        b, hh = k // 2, k % 2
        xk = np.ascontiguousarray(
            xf[b, :, hh * PIX_PER_CORE : (hh + 1) * PIX_PER_CORE]
        ).astype(ml_dtypes.bfloat16)
        in_maps.append({"xin": xk, "wc": wc_aug, "cst": cst4, "ident": ident, "sel": sel})
    return in_maps


def _run(inputs, trace=False):
    from concourse.bass_utils import run_bass_kernel_spmd

    if "nc" not in _cache:
        _cache["nc"] = _build()
    nc = _cache["nc"]

    in_maps = _prep_in_maps(inputs)
    res = run_bass_kernel_spmd(nc, in_maps, list(range(NCORES)), trace=trace)
    out = np.empty((B, O, H * Wd), np.float32)
    for k in range(NCORES):
        b, hh = k // 2, k % 2
        ok = np.asarray(res.results[k]["out"]).astype(np.float32).T  # [64, P]
        out[b, :, hh * PIX_PER_CORE : (hh + 1) * PIX_PER_CORE] = ok
    return out.reshape(B, O, H, Wd), res.exec_time_ns


def kernel(**inputs) -> np.ndarray:
    out, _ = _run(inputs, trace=False)
    return out
